# revision 1
# baseline (speedup 1.0000x reference)
"""Bilinear grid_sample (zeros padding, align_corners=False) Bass kernel for TRN2.

Per-core problem: x [64, H*W] f32 (NCHW flattened), gxy [128, 2*NT] f32
(host-transposed grid planes; cols 0:NT = gx, NT:2NT = gy, where plane[p, t]
= grid[t*128 + p]), out [64, H*W] f32.

Strategy:
  1. Build a "vertical pair" gather table TBL [HW+W+2, 128] fp16 in DRAM:
       TBL[r, 0:64]   = x_nhwc[r - W - 1]   (zeros outside [0, HW))
       TBL[r, 64:128] = x_nhwc[r - 1]       (zeros outside [0, HW))
     built by transposing x tiles on TensorE ([64, 128] -> [128, 64]),
     casting f32->fp16 on DVE, and writing each staged [128, 4, 64] twice
     (lower half at rows 512c+1, upper half at rows 512c+W+1).
  2. Per pixel compute r = (clamp(y0,-1,H-1)+1)*W + clamp(x0,-1,W-1) + 1.
     One indirect-DMA gather of 2 consecutive rows (512 B) per pixel fetches
     all 4 bilinear taps for all 64 channels:
       [r, 0:64]=tap(y0,x0) [r, 64:128]=tap(y1,x0)
       [r+1, 0:64]=tap(y0,x1) [r+1, 64:128]=tap(y1,x1)
  3. Weighted sum on DVE (weights premasked for zeros padding), TensorE
     transpose back to [ch, px], ScalarE PSUM-evict cast fp16->f32, DMA out.
"""

from contextlib import ExitStack

import numpy as np

import concourse.bass as bass
import concourse.tile as tile
from concourse import mybir
from concourse.masks import make_identity

F32 = mybir.dt.float32
F16 = mybir.dt.float16
I32 = mybir.dt.int32
MUL = mybir.AluOpType.mult
ADD = mybir.AluOpType.add
SUB = mybir.AluOpType.subtract
MAX = mybir.AluOpType.max
MIN = mybir.AluOpType.min
GE = mybir.AluOpType.is_ge
LE = mybir.AluOpType.is_le
GT = mybir.AluOpType.is_gt


def gs_body(ctx: ExitStack, tc: tile.TileContext, out_ap, x_ap, gxy_ap, *,
            H=256, W=256, K=16):
    nc = tc.nc
    C = 64
    HW = H * W
    NT = HW // 128            # pixel tiles of 128 (weight-plane columns)
    assert HW % 512 == 0
    NCHUNK = HW // 512        # build chunks of 512 px
    NGATHER = NT // K         # gather iters, K tiles each
    TBL_ROWS = HW + W + 2

    tbl = nc.dram_tensor("tbl", [TBL_ROWS, 2 * C], F16, kind="Internal").ap()

    persist = ctx.enter_context(tc.tile_pool(name="persist", bufs=1))
    loadp = ctx.enter_context(tc.tile_pool(name="loadp", bufs=3))
    psumb = ctx.enter_context(tc.tile_pool(name="psumb", bufs=2, space="PSUM"))
    stage = ctx.enter_context(tc.tile_pool(name="stage", bufs=3))
    gath = ctx.enter_context(tc.tile_pool(name="gath", bufs=2))
    accp = ctx.enter_context(tc.tile_pool(name="accp", bufs=2))
    psumo = ctx.enter_context(tc.tile_pool(name="psumo", bufs=2, space="PSUM"))
    outp = ctx.enter_context(tc.tile_pool(name="outp", bufs=3))

    ident32 = persist.tile([128, 128], F32)
    make_identity(nc, ident32[:])
    ident16 = persist.tile([128, 128], F16)
    make_identity(nc, ident16[:])

    # ---------------- prologue: grid -> weights + gather indices ----------
    g_sb = persist.tile([128, 2 * NT], F32)
    nc.sync.dma_start(g_sb[:], gxy_ap[:])

    def axis_prep(gsl, size, ax):
        """Return (frac t, wm0=(1-t)*valid0, wm1=t*valid1, clamped floor)."""
        def ptile(dt, name):
            return persist.tile([128, NT], dt, name=f"{name}_{ax}",
                                tag=f"{name}_{ax}")
        v = ptile(F32, "v")
        # unnormalize: ((g+1)*size - 1)/2 = g*(size/2) + (size-1)/2
        nc.vector.tensor_scalar(v[:], gsl, size / 2.0, (size - 1) / 2.0, MUL, ADD)
        vi = ptile(I32, "vi")
        nc.vector.tensor_copy(vi[:], v[:])          # cast, rounding unknown
        vf = ptile(F32, "vf")
        nc.vector.tensor_copy(vf[:], vi[:])         # exact back-cast
        adj = ptile(F32, "adj")
        nc.vector.tensor_tensor(adj[:], vf[:], v[:], op=GT)  # 1.0 if vf > v
        nc.vector.tensor_tensor(vf[:], vf[:], adj[:], op=SUB)  # floor(v)
        t = ptile(F32, "t")
        nc.vector.tensor_tensor(t[:], v[:], vf[:], op=SUB)     # frac in [0,1)
        m0a = ptile(F32, "m0a")
        nc.vector.tensor_scalar(m0a[:], vf[:], 0.0, None, GE)
        m0b = ptile(F32, "m0b")
        nc.vector.tensor_scalar(m0b[:], vf[:], size - 1.0, None, LE)
        nc.vector.tensor_tensor(m0a[:], m0a[:], m0b[:], op=MUL)  # valid0
        m1a = ptile(F32, "m1a")
        nc.vector.tensor_scalar(m1a[:], vf[:], -1.0, None, GE)
        m1b = ptile(F32, "m1b")
        nc.vector.tensor_scalar(m1b[:], vf[:], size - 2.0, None, LE)
        nc.vector.tensor_tensor(m1a[:], m1a[:], m1b[:], op=MUL)  # valid1
        # wm0 = (1 - t) * valid0 ; wm1 = t * valid1
        wm0 = ptile(F32, "wm0")
        nc.vector.tensor_scalar(wm0[:], t[:], -1.0, 1.0, MUL, ADD)
        nc.vector.tensor_tensor(wm0[:], wm0[:], m0a[:], op=MUL)
        nc.vector.tensor_tensor(t[:], t[:], m1a[:], op=MUL)      # t <- wm1
        # clamped floor for addressing
        nc.vector.tensor_scalar(vf[:], vf[:], -1.0, size - 1.0, MAX, MIN)
        return wm0, t, vf

    wx0, wx1, xc = axis_prep(g_sb[:, 0:NT], float(W), "x")
    wy0, wy1, yc = axis_prep(g_sb[:, NT:2 * NT], float(H), "y")

    # combined weights, interleaved [p, (k t)] fp16 with t in (00,10,01,11)
    # order matching gathered layout [r:upper, r:lower, r+1:upper, r+1:lower]
    wcomb = persist.tile([128, NT * 4], F16)
    wv = wcomb[:].rearrange("p (k t) -> p k t", t=4)
    wtmp = persist.tile([128, NT], F32)
    for ti, (wy, wx) in enumerate(((wy0, wx0), (wy1, wx0), (wy0, wx1), (wy1, wx1))):
        nc.vector.tensor_tensor(wtmp[:], wy[:], wx[:], op=MUL)
        nc.vector.tensor_copy(wv[:, :, ti:ti + 1].squeeze(2), wtmp[:])

    # gather row index r = yc*W + xc + (W + 1), exact small ints in f32
    rf = persist.tile([128, NT], F32)
    nc.vector.tensor_scalar(rf[:], yc[:], float(W), W + 1.0, MUL, ADD)
    nc.vector.tensor_tensor(rf[:], rf[:], xc[:], op=ADD)
    idx = persist.tile([128, NT], I32)
    nc.vector.tensor_copy(idx[:], rf[:])

    # ---------------- zero the table edge rows ----------------------------
    zero_sb = persist.tile([128, 2 * C], F16)
    nc.gpsimd.memset(zero_sb[:], 0.0)

    def zero_rows(r0, r1):
        n = r1 - r0
        while n > 0:
            step = min(n, 128)
            nc.sync.dma_start(tbl[r0:r0 + step, :], zero_sb[0:step, :])
            r0 += step
            n -= step

    zero_rows(0, W + 1)                 # head: covers unused uppers + row 0
    zero_rows(HW + 1, HW + W + 2)       # tail: unused lowers + final row

    # ---------------- build the gather table ------------------------------
    for c in range(NCHUNK):
        xs = loadp.tile([C, 512], F32)
        nc.sync.dma_start(xs[:], x_ap[:, 512 * c:512 * (c + 1)])
        pt = psumb.tile([128, 4 * C], F32)
        for j in range(4):
            nc.tensor.transpose(pt[:, C * j:C * (j + 1)],
                                xs[:, 128 * j:128 * (j + 1)],
                                ident32[0:C, 0:C])
        st = stage.tile([128, 4 * C], F16)
        nc.vector.tensor_copy(st[:], pt[:])
        # lower halves at rows 512c+1, upper halves W rows later
        lo = tbl[512 * c + 1:512 * c + 513, C:2 * C]
        up = tbl[512 * c + W + 1:512 * c + W + 513, 0:C]
        stv = st[:].rearrange("p (j ch) -> p j ch", j=4)
        nc.sync.dma_start(lo.rearrange("(j p) ch -> p j ch", j=4), stv)
        nc.sync.dma_start(up.rearrange("(j p) ch -> p j ch", j=4), stv)

    # ---------------- gather + weighted sum + transpose out ---------------
    for g in range(NGATHER):
        gb = gath.tile([128, K * 4 * C], F16)
        # HW indirect DMA consumes ONE offset per partition and fetches a
        # contiguous [free_size] block, so issue one gather per 128-px tile.
        for k in range(K):
            nc.gpsimd.indirect_dma_start(
                out=gb[:, 4 * C * k:4 * C * (k + 1)],
                out_offset=None,
                in_=tbl[:],
                in_offset=bass.IndirectOffsetOnAxis(
                    ap=idx[:, K * g + k:K * g + k + 1], axis=0),
            )
        gb4 = gb[:].rearrange("p (k t ch) -> p k t ch", k=K, t=4)
        wsl = wcomb[:, 4 * K * g:4 * K * (g + 1)]
        wb = wsl.rearrange("p (k t) -> p k t", t=4)
        acc = accp.tile([128, K * C], F16)
        tmp = accp.tile([128, K * C], F16)
        accv = acc[:].rearrange("p (k ch) -> p k ch", k=K)
        tmpv = tmp[:].rearrange("p (k ch) -> p k ch", k=K)
        for ti in range(4):
            dst = accv if ti == 0 else tmpv
            nc.vector.tensor_tensor(
                dst,
                gb4[:, :, ti:ti + 1, :].squeeze(2),
                wb[:, :, ti:ti + 1].to_broadcast([128, K, C]),
                op=MUL,
            )
            if ti > 0:
                nc.vector.tensor_tensor(accv, accv, tmpv, op=ADD)
        po = psumo.tile([C, K * 128], F16)
        for t in range(K):
            nc.tensor.transpose(po[:, 128 * t:128 * (t + 1)],
                                acc[:, C * t:C * (t + 1)],
                                ident16[:])
        ob = outp.tile([C, K * 128], F32)
        nc.scalar.activation(ob[:], po[:], mybir.ActivationFunctionType.Copy)
        nc.sync.dma_start(out_ap[:, 128 * K * g:128 * K * (g + 1)], ob[:])


def host_prep_gxy(grid_flat):
    """grid_flat [HW, 2] f32 -> [128, 2*NT] f32 (gx plane | gy plane)."""
    HW = grid_flat.shape[0]
    NT = HW // 128
    g = grid_flat.reshape(NT, 128, 2)
    return np.ascontiguousarray(
        np.concatenate([g[:, :, 0].T, g[:, :, 1].T], axis=1))




# ----------------------------------------------------------------------------
# self-contained kernel entry point
# ----------------------------------------------------------------------------
import concourse.bacc as bacc
from concourse.bass_utils import run_bass_kernel_spmd

N_CORES = 8
H = W = 256
C = 64
HW = H * W
K = 16

_NC = None
LAST_RESULT = None


def _build_nc():
    global _NC
    if _NC is not None:
        return _NC
    nc = bacc.Bacc("TRN2", target_bir_lowering=False, debug=False)
    x = nc.dram_tensor("x", [C, HW], F32, kind="ExternalInput").ap()
    gxy = nc.dram_tensor("gxy", [128, 2 * (HW // 128)], F32,
                         kind="ExternalInput").ap()
    out = nc.dram_tensor("out", [C, HW], F32, kind="ExternalOutput").ap()
    with tile.TileContext(nc) as tc, ExitStack() as ctx:
        gs_body(ctx, tc, out, x, gxy, H=H, W=W, K=K)
    nc.compile()
    _NC = nc
    return nc


def kernel(x, grid, trace=False):
    global LAST_RESULT
    x = np.asarray(x, dtype=np.float32)
    grid = np.asarray(grid, dtype=np.float32)
    assert x.shape == (N_CORES, C, H, W) and grid.shape == (N_CORES, H, W, 2)
    nc = _build_nc()
    in_maps = []
    for n in range(N_CORES):
        in_maps.append({
            "x": np.ascontiguousarray(x[n].reshape(C, HW)),
            "gxy": host_prep_gxy(grid[n].reshape(HW, 2)),
        })
    res = run_bass_kernel_spmd(nc, in_maps, core_ids=list(range(N_CORES)),
                               trace=trace)
    LAST_RESULT = res
    out = np.stack([m["out"] for m in res.results])
    return out.reshape(N_CORES, C, H, W)



# revision 9
# speedup vs baseline: 1.3132x; 1.3132x over previous
"""Bilinear grid_sample (zeros padding, align_corners=False) Bass kernel, v3.

Per-core problem: x [64, H*W] f32 (NCHW flattened), gxy [128, 2*NT] f32
(host-transposed grid planes), out [64, H*W] f32.

v3 design (vs v1's 512 indirect DMAs at ~1.2us SWDGE fixed cost each):

Table: T[rho] = [x[(rho-257) mod HW] | x[(rho-1) mod HW]] fp16, rho in
[0, HW+1).  The mod-HW fold aliases the head rows (lo-half live, up-half
weight-0 for every reader) with the tail rows (up-half live, lo-half
weight-0), so the whole table fits in HW+1 rows and every row holds finite
x data (no NaN-able gaps; out-of-bounds taps are killed by premasked
weights, as before).

Gather: ONE dma_gather (GPSIMD ucode, ~1us + 0.34ns/idx) per 2048 px
instead of 16 indirect DMAs.  int16 indices address 512-B-stride windows:
idx = rho >> 1, each window reads 3 rows (768 B) covering rows rho, rho+1
for either parity; idx <= 32767 fits int16.  The window's 6 taps are
weighted with parity-masked weights (w6), so the junk row contributes 0.

Build: 8 chunks of 8192 px.  SWDGE cast-load f32->f16, 64 TensorE fp16
transposes per chunk with stride-64 px grouping so SBUF partition p holds
table rows [8192c + 64p + 1, +64) consecutively, up-half via a
partition-shift-by-4 SBUF->SBUF DMA, then ONE table-write DMA per chunk
(128 x 16 KB contiguous descriptors).  Rows [0, 257) up-halves come from
the image tail (the fold) and are patched after chunk 7.

Weighted sum: ScalarE expands w6 [128, t*6] -> packed [128, t*6*64] (ACT
is otherwise idle; a DVE broadcast-multiply would run at 1 el/cycle),
then DVE does one packed 2x multiply + 3 adds.  TensorE transposes back
to [ch, px], ScalarE PSUM-evict casts fp16->f32, HWDGE DMA out.
"""

from contextlib import ExitStack

import numpy as np

import concourse.bass as bass
import concourse.tile as tile
from concourse import mybir
from concourse.masks import make_identity

F32 = mybir.dt.float32
F16 = mybir.dt.float16
I16 = mybir.dt.int16
I32 = mybir.dt.int32
MUL = mybir.AluOpType.mult
ADD = mybir.AluOpType.add
SUB = mybir.AluOpType.subtract
MAX = mybir.AluOpType.max
MIN = mybir.AluOpType.min
GE = mybir.AluOpType.is_ge
LE = mybir.AluOpType.is_le
GT = mybir.AluOpType.is_gt


def gs_body(ctx: ExitStack, tc: tile.TileContext, out_ap, x_ap, gxy_ap, *,
            H=256, W=256, PX_PER_CALL=1024):
    nc = tc.nc
    C = 64
    HW = H * W
    NT = HW // 128              # 128-px tiles
    NCALL = HW // PX_PER_CALL   # dma_gather calls
    TPC = PX_PER_CALL // 128    # px tiles per call
    NW = HW // 2                # gather windows (idx = rho >> 1)
    EL = 384                    # window elements (3 rows x 128 fp16)
    STEP = 256                  # window stride in elements (2 rows)

    CHUNK = 8192                # build chunk (px)
    NCH = HW // CHUNK
    PPP = CHUNK // 128          # rows per partition per chunk (64)
    SH = (W + 1) // PPP         # lo->up partition shift (4)
    OV = W + 1 - SH * PPP       # leftover absorbed by the +1 dest offset
    assert OV == 1

    tblh = nc.dram_tensor("tbl", [HW + 1, 2 * C], F16, kind="Internal")
    tbl = tblh.ap()

    persist = ctx.enter_context(tc.tile_pool(name="persist", bufs=1))

    ident16 = persist.tile([128, 128], F16)
    make_identity(nc, ident16[:])

    # ---------------- prologue: grid -> weights + gather indices ----------
    wcomb = persist.tile([128, NT * 6], F16)
    idx16 = persist.tile([128, 8 * NT], I16)

    pctx = ExitStack()
    prolp = pctx.enter_context(tc.tile_pool(name="prolp", bufs=1))
    g_sb = prolp.tile([128, 2 * NT], F32)
    nc.sync.dma_start(g_sb[:], gxy_ap[:])

    def ptile(dt, name):
        return prolp.tile([128, NT], dt, name=name, tag=name)

    def axis_prep(gsl, size, ax):
        """Return (wm0=(1-t)*valid0, wm1=t*valid1, clamped floor)."""
        def p(dt, name):
            return ptile(dt, f"{name}_{ax}")
        v = p(F32, "v")
        nc.vector.tensor_scalar(v[:], gsl, size / 2.0, (size - 1) / 2.0, MUL, ADD)
        vi = p(I32, "vi")
        nc.vector.tensor_copy(vi[:], v[:])          # cast, rounding unknown
        vf = p(F32, "vf")
        nc.vector.tensor_copy(vf[:], vi[:])         # exact back-cast
        adj = p(F32, "adj")
        nc.vector.tensor_tensor(adj[:], vf[:], v[:], op=GT)  # 1.0 if vf > v
        nc.vector.tensor_tensor(vf[:], vf[:], adj[:], op=SUB)  # floor(v)
        t = p(F32, "t")
        nc.vector.tensor_tensor(t[:], v[:], vf[:], op=SUB)     # frac in [0,1)
        m0a = p(F32, "m0a")
        nc.vector.tensor_scalar(m0a[:], vf[:], 0.0, None, GE)
        m0b = p(F32, "m0b")
        nc.vector.tensor_scalar(m0b[:], vf[:], size - 1.0, None, LE)
        nc.vector.tensor_tensor(m0a[:], m0a[:], m0b[:], op=MUL)  # valid0
        m1a = p(F32, "m1a")
        nc.vector.tensor_scalar(m1a[:], vf[:], -1.0, None, GE)
        m1b = p(F32, "m1b")
        nc.vector.tensor_scalar(m1b[:], vf[:], size - 2.0, None, LE)
        nc.vector.tensor_tensor(m1a[:], m1a[:], m1b[:], op=MUL)  # valid1
        wm0 = p(F32, "wm0")
        nc.vector.tensor_scalar(wm0[:], t[:], -1.0, 1.0, MUL, ADD)
        nc.vector.tensor_tensor(wm0[:], wm0[:], m0a[:], op=MUL)
        nc.vector.tensor_tensor(t[:], t[:], m1a[:], op=MUL)      # t <- wm1
        nc.vector.tensor_scalar(vf[:], vf[:], -1.0, size - 1.0, MAX, MIN)
        return wm0, t, vf

    wx0, wx1, xc = axis_prep(g_sb[:, 0:NT], float(W), "x")
    wy0, wy1, yc = axis_prep(g_sb[:, NT:2 * NT], float(H), "y")

    # r = yc*W + xc + (W+1) in [0, HW + W + 1); fold rho = r mod HW
    rf = ptile(F32, "rf")
    nc.vector.tensor_scalar(rf[:], yc[:], float(W), W + 1.0, MUL, ADD)
    nc.vector.tensor_tensor(rf[:], rf[:], xc[:], op=ADD)
    fold = ptile(F32, "fold")
    nc.vector.tensor_scalar(fold[:], rf[:], HW - 0.5, None, GE)  # r >= HW
    nc.vector.tensor_scalar(fold[:], fold[:], float(HW), None, MUL)
    nc.vector.tensor_tensor(rf[:], rf[:], fold[:], op=SUB)       # rho
    # q = rho >> 1 (exact floor), par = rho - 2q
    qf = ptile(F32, "qf")
    nc.vector.tensor_scalar(qf[:], rf[:], 0.5, None, MUL)
    qi = ptile(I32, "qi")
    nc.vector.tensor_copy(qi[:], qf[:])
    qr = ptile(F32, "qr")
    nc.vector.tensor_copy(qr[:], qi[:])
    adj2 = ptile(F32, "adj2")
    nc.vector.tensor_tensor(adj2[:], qr[:], qf[:], op=GT)
    nc.vector.tensor_tensor(qr[:], qr[:], adj2[:], op=SUB)       # q
    par = ptile(F32, "par")
    nc.vector.tensor_scalar(par[:], qr[:], -2.0, None, MUL)
    nc.vector.tensor_tensor(par[:], par[:], rf[:], op=ADD)       # parity
    even = ptile(F32, "even")
    nc.vector.tensor_scalar(even[:], par[:], -1.0, 1.0, MUL, ADD)  # 1-par

    # 6 window-tap weights, interleaved [p, (k s)], s = (m0u,m0l,m1u,m1l,m2u,m2l)
    #   m0: even * (w00, w10); m2: par * (w01, w11)
    #   m1: even * (w01, w11) + par * (w00, w10)
    wv = wcomb[:].rearrange("p (k s) -> p k s", s=6)
    wa = ptile(F32, "wa")
    wb = ptile(F32, "wb")
    wtmp = ptile(F32, "wtmp")
    for h, (wyA, wyB) in enumerate(((wy0, wy1),)):
        pass
    for half, (wy,) in enumerate(((wy0,), (wy1,))):
        # w_y0x0 / w_y1x0 -> taps at column x0 ; w_y0x1 / w_y1x1 at x1
        nc.vector.tensor_tensor(wa[:], wy[:], wx0[:], op=MUL)   # w(y,x0)
        nc.vector.tensor_tensor(wb[:], wy[:], wx1[:], op=MUL)   # w(y,x1)
        # m0 = even * w(y,x0)
        nc.vector.tensor_tensor(wtmp[:], wa[:], even[:], op=MUL)
        nc.vector.tensor_copy(wv[:, :, half:half + 1].squeeze(2), wtmp[:])
        # m2 = par * w(y,x1)
        nc.vector.tensor_tensor(wtmp[:], wb[:], par[:], op=MUL)
        nc.vector.tensor_copy(wv[:, :, 4 + half:5 + half].squeeze(2), wtmp[:])
        # m1 = even * w(y,x1) + par * w(y,x0)
        nc.vector.tensor_tensor(wtmp[:], wb[:], even[:], op=MUL)
        nc.vector.tensor_tensor(wa[:], wa[:], par[:], op=MUL)
        nc.vector.tensor_tensor(wtmp[:], wtmp[:], wa[:], op=ADD)
        nc.vector.tensor_copy(wv[:, :, 2 + half:3 + half].squeeze(2), wtmp[:])

    # ---------------- idx: [128, NT] f32 q -> [128, HW/16] i16 wrapped ----
    idxA = prolp.tile([16, 8 * NT], F32)        # [b, NT*a + j] = q[16a+b, j]
    for a in range(8):
        nc.sync.dma_start(idxA[:, NT * a:NT * (a + 1)],
                          qr[16 * a:16 * a + 16, :])
    nc.vector.tensor_copy(
        idx16[0:16, :].rearrange("p (j a) -> p a j", a=8),
        idxA[:].rearrange("p (a j) -> p a j", j=NT))
    for g in range(1, 8):
        nc.sync.dma_start(idx16[16 * g:16 * g + 16, :], idx16[0:16, :])
    pctx.close()

    # ---------------- build the gather table ------------------------------
    with ExitStack() as bctx:
        loadp = bctx.enter_context(tc.tile_pool(name="loadp", bufs=2))
        psumb = bctx.enter_context(tc.tile_pool(name="psumb", bufs=4,
                                                space="PSUM"))
        xtp = bctx.enter_context(tc.tile_pool(name="xtp", bufs=2))
        xtsp = bctx.enter_context(tc.tile_pool(name="xtsp", bufs=2))
        stagep = bctx.enter_context(tc.tile_pool(name="stagep", bufs=2))
        xt_prev = None
        xt_last = None
        for c in range(NCH):
            xc_t = loadp.tile([C, CHUNK], F16)
            nc.gpsimd.dma_start(xc_t[:], x_ap[:, CHUNK * c:CHUNK * (c + 1)])
            xcv = xc_t[:].rearrange("c (a j) -> c a j", j=PPP)
            xt = xtp.tile([128, PPP * C], F16)
            for q in range(PPP // 16):
                pt = psumb.tile([128, 16 * C], F16)
                for jj in range(16):
                    j = 16 * q + jj
                    nc.tensor.transpose(pt[:, C * jj:C * (jj + 1)],
                                        xcv[:, :, j:j + 1].squeeze(2),
                                        ident16[0:C, 0:C])
                nc.vector.tensor_copy(xt[:, 16 * C * q:16 * C * (q + 1)], pt[:])
            xts = xtsp.tile([128, PPP * C], F16)
            nc.sync.dma_start(xts[SH:128, :], xt[0:128 - SH, :])
            if xt_prev is None:
                nc.gpsimd.memset(xts[0:SH, :], 0.0)   # patched after chunk 7
            else:
                nc.sync.dma_start(xts[0:SH, :], xt_prev[128 - SH:128, :])
            xt_prev = xt

            stage = stagep.tile([128, PPP * 2 * C], F16)
            sv = stage[:].rearrange("p (j h) -> p j h", h=2 * C)
            xtv = xt[:].rearrange("p (j ch) -> p j ch", ch=C)
            xtsv = xts[:].rearrange("p (j ch) -> p j ch", ch=C)
            nc.vector.tensor_copy(sv[:, :, 0:C], xtsv[:, :, :])
            nc.vector.tensor_copy(sv[:, :, C:2 * C], xtv[:, :, :])
            dst = tbl[CHUNK * c + 1:CHUNK * c + 1 + CHUNK, :]
            nc.sync.dma_start(dst.rearrange("(p j) h -> p (j h)", p=128),
                              stage[:])
            xt_last = xt

        # patches from the image tail (the fold):
        # rows [1, 257) up-half = x[65280 + (row-1)] = xt7[124 + p, j]
        updst = tbl[1:1 + SH * PPP, 0:C]
        nc.sync.dma_start(
            updst.rearrange("(p j) h -> p j h", p=SH),
            xt_last[128 - SH:128, :].rearrange("p (j ch) -> p j ch", ch=C))
        # row 0 = [x[HW-W-1] | x[HW-1]]
        nc.sync.dma_start(tbl[0:1, 0:C],
                          xt_last[123:124, (PPP - 1) * C:PPP * C])
        nc.sync.dma_start(tbl[0:1, C:2 * C],
                          xt_last[127:128, (PPP - 1) * C:PPP * C])

    # ---------------- gather + weighted sum + transpose out ---------------
    gathp = ctx.enter_context(tc.tile_pool(name="gathp", bufs=2))
    wexp = ctx.enter_context(tc.tile_pool(name="wexp", bufs=2))
    accp = ctx.enter_context(tc.tile_pool(name="accp", bufs=2))
    psumo = ctx.enter_context(tc.tile_pool(name="psumo", bufs=4, space="PSUM"))
    outp = ctx.enter_context(tc.tile_pool(name="outp", bufs=2))

    win_ap = bass.AP(tblh, 0, [[STEP, NW], [1, EL]])
    ICOLS = PX_PER_CALL // 16

    for g in range(NCALL):
        gb = gathp.tile([128, TPC * EL], F16)
        nc.gpsimd.dma_gather(
            gb[:].rearrange("p (t e) -> p t e", e=EL),
            win_ap,
            idx16[:, ICOLS * g:ICOLS * (g + 1)],
            PX_PER_CALL,
            PX_PER_CALL,
            EL,
            elem_step=STEP,
        )
        # expand w6 on ScalarE (ACT handles the broadcast; DVE would run 1x)
        wt = wexp.tile([128, TPC * 6 * C], F16)
        wslice = wcomb[:, 6 * TPC * g:6 * TPC * (g + 1)]
        nc.scalar.activation(
            wt[:].rearrange("p (ks ch) -> p ks ch", ch=C),
            wslice.unsqueeze(2).to_broadcast([128, 6 * TPC, C]),
            mybir.ActivationFunctionType.Copy)
        nc.vector.tensor_tensor(gb[:], gb[:], wt[:], op=MUL)
        # sum 6 taps: (m0 + m1) + m2, then the up/lo halves
        gbm = gb[:].rearrange("p (t m d) -> p t m d", m=3, d=2 * C)
        acc2 = accp.tile([128, TPC * 2 * C], F16)
        a2 = acc2[:].rearrange("p (t d) -> p t d", d=2 * C)
        nc.vector.tensor_tensor(a2, gbm[:, :, 0:1, :].squeeze(2),
                                gbm[:, :, 1:2, :].squeeze(2), op=ADD)
        nc.vector.tensor_tensor(a2, a2, gbm[:, :, 2:3, :].squeeze(2), op=ADD)
        acc = accp.tile([128, TPC * C], F16)
        av = acc[:].rearrange("p (t ch) -> p t ch", ch=C)
        a2v = acc2[:].rearrange("p (t h ch) -> p t h ch", h=2, ch=C)
        nc.vector.tensor_tensor(av, a2v[:, :, 0:1, :].squeeze(2),
                                a2v[:, :, 1:2, :].squeeze(2), op=ADD)
        # transpose back to [ch, px] + evict with f32 cast
        ob = outp.tile([C, TPC * 128], F32)
        for q in range(TPC // 8):
            po = psumo.tile([C, 8 * 128], F16)
            for tt in range(8):
                t = 8 * q + tt
                nc.tensor.transpose(po[:, 128 * tt:128 * (tt + 1)],
                                    acc[:, C * t:C * (t + 1)],
                                    ident16[:])
            nc.scalar.activation(ob[:, 1024 * q:1024 * (q + 1)], po[:],
                                 mybir.ActivationFunctionType.Copy)
        nc.sync.dma_start(
            out_ap[:, PX_PER_CALL * g:PX_PER_CALL * (g + 1)], ob[:])


def host_prep_gxy(grid_flat):
    """grid_flat [HW, 2] f32 -> [128, 2*NT] f32 (gx plane | gy plane)."""
    HW = grid_flat.shape[0]
    NT = HW // 128
    g = grid_flat.reshape(NT, 128, 2)
    return np.ascontiguousarray(
        np.concatenate([g[:, :, 0].T, g[:, :, 1].T], axis=1))


# ----------------------------------------------------------------------------
# self-contained kernel entry point
# ----------------------------------------------------------------------------
import concourse.bacc as bacc
from concourse.bass_utils import run_bass_kernel_spmd

N_CORES = 8
H = W = 256
C = 64
HW = H * W

_NC = None
LAST_RESULT = None


def _build_nc():
    global _NC
    if _NC is not None:
        return _NC
    nc = bacc.Bacc("TRN2", target_bir_lowering=False, debug=False)
    x = nc.dram_tensor("x", [C, HW], F32, kind="ExternalInput").ap()
    gxy = nc.dram_tensor("gxy", [128, 2 * (HW // 128)], F32,
                         kind="ExternalInput").ap()
    out = nc.dram_tensor("out", [C, HW], F32, kind="ExternalOutput").ap()
    with tile.TileContext(nc) as tc, ExitStack() as ctx:
        gs_body(ctx, tc, out, x, gxy, H=H, W=W)
    nc.compile()
    _NC = nc
    return nc


def kernel(x, grid, trace=False):
    global LAST_RESULT
    x = np.asarray(x, dtype=np.float32)
    grid = np.asarray(grid, dtype=np.float32)
    assert x.shape == (N_CORES, C, H, W) and grid.shape == (N_CORES, H, W, 2)
    nc = _build_nc()
    in_maps = []
    for n in range(N_CORES):
        in_maps.append({
            "x": np.ascontiguousarray(x[n].reshape(C, HW)),
            "gxy": host_prep_gxy(grid[n].reshape(HW, 2)),
        })
    res = run_bass_kernel_spmd(nc, in_maps, core_ids=list(range(N_CORES)),
                               trace=trace)
    LAST_RESULT = res
    out = np.stack([m["out"] for m in res.results])
    return out.reshape(N_CORES, C, H, W)
